# revision 4
# baseline (speedup 1.0000x reference)
"""Episodic-memory retrieval kernel for 8 Trainium2 NeuronCores.

Pipeline (classic sharded ANN retrieval, capacity-axis sharding):
  K1: per-core cosine sim (queries x embedding-shard) + local top-8
  host: merge candidates -> global top-8, build gather/scatter indices
  K2: per-core indirect-DMA gather of owned winners from episode shards,
      scatter into a compacted per-core output; host reassembles.
"""

import numpy as np

import concourse.bass as bass
import concourse.bacc as bacc
import concourse.mybir as mybir
from concourse.bass import IndirectOffsetOnAxis
from concourse.bass_utils import run_bass_kernel_spmd
from concourse.masks import make_identity
from concourse.tile import TileContext

F32 = mybir.dt.float32
I32 = mybir.dt.int32
U32 = mybir.dt.uint32

B, H, C, S, CS, K = 256, 256, 16384, 64, 32, 8
NCORES = 8
CSH = C // NCORES        # 2048 episodes per core shard
EPS = 1e-8
NB = 4                   # K2 batches of 128 winner-slots -> capacity 512/core
CAP = NB * 128
ROW = S * H              # 16384 f32 per full episode
CROW = CS * H            # 8192 f32 per compressed episode

_progs = {}


def _build_k1():
    nc = bacc.Bacc(None, target_bir_lowering=False)
    q = nc.dram_tensor("q", [B, H], F32, kind="ExternalInput")
    e = nc.dram_tensor("e", [CSH, H], F32, kind="ExternalInput")
    scores = nc.dram_tensor("scores", [B, K], F32, kind="ExternalOutput")
    idx = nc.dram_tensor("idx", [B, K], U32, kind="ExternalOutput")

    with TileContext(nc) as tc:
        with (
            tc.tile_pool(name="work", bufs=3) as wp,
            tc.tile_pool(name="psum", bufs=4, space="PSUM") as pp,
            tc.tile_pool(name="persist", bufs=1) as keep,
        ):
            ident = keep.tile([128, 128], F32, tag="ident")
            make_identity(nc, ident[:])

            def normalize(t):
                # t: [128, H] rows; t <- t / max(||t||, EPS) per row
                sq = wp.tile([128, H], F32, tag="sq")
                ssq = wp.tile([128, 1], F32, tag="ssq")
                nc.scalar.activation(
                    out=sq[:], in_=t[:],
                    func=mybir.ActivationFunctionType.Square,
                    accum_out=ssq[:],
                )
                nrm = wp.tile([128, 1], F32, tag="nrm")
                nc.scalar.activation(
                    out=nrm[:], in_=ssq[:],
                    func=mybir.ActivationFunctionType.Sqrt,
                )
                nc.vector.tensor_scalar_max(nrm[:], nrm[:], EPS)
                rn = wp.tile([128, 1], F32, tag="rn")
                nc.vector.reciprocal(rn[:], nrm[:])
                nc.vector.tensor_scalar_mul(t[:], t[:], rn[:])

            # qT[kt] : [128 (h), B] with h-slice kt
            qT = [keep.tile([128, B], F32, tag=f"qT{i}", name=f"qT{i}") for i in range(H // 128)]
            for mt in range(B // 128):
                qt = wp.tile([128, H], F32, tag="qt")
                nc.sync.dma_start(out=qt[:], in_=q[mt * 128:(mt + 1) * 128, :])
                normalize(qt)
                for kt in range(H // 128):
                    pt = pp.tile([128, 128], F32, space="PSUM", tag="pt")
                    nc.tensor.transpose(
                        out=pt[:], in_=qt[:, kt * 128:(kt + 1) * 128],
                        identity=ident[:],
                    )
                    nc.vector.tensor_copy(
                        out=qT[kt][:, mt * 128:(mt + 1) * 128], in_=pt[:]
                    )

            # eT[kt] : [128 (h), CSH]
            eT = [keep.tile([128, CSH], F32, tag=f"eT{i}", name=f"eT{i}") for i in range(H // 128)]
            for ct in range(CSH // 128):
                et = wp.tile([128, H], F32, tag="et")
                nc.sync.dma_start(out=et[:], in_=e[ct * 128:(ct + 1) * 128, :])
                normalize(et)
                for kt in range(H // 128):
                    pt = pp.tile([128, 128], F32, space="PSUM", tag="pt")
                    nc.tensor.transpose(
                        out=pt[:], in_=et[:, kt * 128:(kt + 1) * 128],
                        identity=ident[:],
                    )
                    nc.vector.tensor_copy(
                        out=eT[kt][:, ct * 128:(ct + 1) * 128], in_=pt[:]
                    )

            # sim[mt] : [128 (query), CSH], then top-8 per row
            for mt in range(B // 128):
                simt = keep.tile([128, CSH], F32, tag=f"sim{mt}", name=f"sim{mt}")
                for nt in range(CSH // 512):
                    ps = pp.tile([128, 512], F32, space="PSUM", tag="ps")
                    for kt in range(H // 128):
                        nc.tensor.matmul(
                            out=ps[:],
                            lhsT=qT[kt][:, mt * 128:(mt + 1) * 128],
                            rhs=eT[kt][:, nt * 512:(nt + 1) * 512],
                            start=(kt == 0),
                            stop=(kt == H // 128 - 1),
                        )
                    nc.vector.tensor_copy(
                        out=simt[:, nt * 512:(nt + 1) * 512], in_=ps[:]
                    )
                s8 = wp.tile([128, 8], F32, tag="s8")
                nc.vector.max(out=s8[:], in_=simt[:])
                i8 = wp.tile([128, 8], U32, tag="i8")
                nc.vector.max_index(out=i8[:], in_max=s8[:], in_values=simt[:])
                nc.sync.dma_start(
                    out=scores[mt * 128:(mt + 1) * 128, :], in_=s8[:]
                )
                nc.sync.dma_start(out=idx[mt * 128:(mt + 1) * 128, :], in_=i8[:])

    nc.compile()
    return nc


def _build_k2():
    # tbl rows: [0, 2*CSH) = episode half-rows (2i, 2i+1), [2*CSH, 3*CSH) =
    # compressed rows. One gather per winner half, two independent
    # gather->scatter chains per batch.
    nc = bacc.Bacc(None, target_bir_lowering=False)
    tbl = nc.dram_tensor("tbl", [3 * CSH, CROW], F32, kind="ExternalInput")
    g0 = nc.dram_tensor("g0", [128, NB], I32, kind="ExternalInput")
    g1 = nc.dram_tensor("g1", [128, NB], I32, kind="ExternalInput")
    slot0 = nc.dram_tensor("slot0", [128, NB], I32, kind="ExternalInput")
    slot1 = nc.dram_tensor("slot1", [128, NB], I32, kind="ExternalInput")
    out = nc.dram_tensor("out", [CAP, ROW], F32, kind="ExternalOutput")
    out_half = out[:].rearrange("w (h x) -> (w h) x", h=2)  # [2*CAP, CROW]

    with TileContext(nc) as tc:
        with (
            tc.tile_pool(name="d0", bufs=3) as p0,
            tc.tile_pool(name="d1", bufs=3) as p1,
            tc.tile_pool(name="idxp", bufs=1) as ip,
        ):
            g0t = ip.tile([128, NB], I32, tag="g0t")
            g1t = ip.tile([128, NB], I32, tag="g1t")
            s0t = ip.tile([128, NB], I32, tag="s0t")
            s1t = ip.tile([128, NB], I32, tag="s1t")
            nc.sync.dma_start(out=g0t[:], in_=g0[:])
            nc.sync.dma_start(out=g1t[:], in_=g1[:])
            nc.sync.dma_start(out=s0t[:], in_=slot0[:])
            nc.sync.dma_start(out=s1t[:], in_=slot1[:])

            for bt in range(NB):
                t0 = p0.tile([128, CROW], F32, tag="t0")
                t1 = p1.tile([128, CROW], F32, tag="t1")
                # half0 of every owned winner (episode half or compressed row)
                nc.gpsimd.indirect_dma_start(
                    out=t0[:],
                    out_offset=None,
                    in_=tbl[:],
                    in_offset=IndirectOffsetOnAxis(ap=g0t[:, bt:bt + 1], axis=0),
                    bounds_check=3 * CSH - 1,
                    oob_is_err=False,
                )
                # half1 (episode half-row 2i+1) for non-compressed winners
                nc.gpsimd.indirect_dma_start(
                    out=t1[:],
                    out_offset=None,
                    in_=tbl[:],
                    in_offset=IndirectOffsetOnAxis(ap=g1t[:, bt:bt + 1], axis=0),
                    bounds_check=3 * CSH - 1,
                    oob_is_err=False,
                )
                nc.gpsimd.indirect_dma_start(
                    out=out_half,
                    out_offset=IndirectOffsetOnAxis(ap=s0t[:, bt:bt + 1], axis=0),
                    in_=t0[:],
                    in_offset=None,
                    bounds_check=2 * CAP - 1,
                    oob_is_err=False,
                )
                # compressed winners' half1 stays zero (pre-zeroed output)
                nc.gpsimd.indirect_dma_start(
                    out=out_half,
                    out_offset=IndirectOffsetOnAxis(ap=s1t[:, bt:bt + 1], axis=0),
                    in_=t1[:],
                    in_offset=None,
                    bounds_check=2 * CAP - 1,
                    oob_is_err=False,
                )

    nc.compile()
    return nc


def _get(name):
    if name not in _progs:
        _progs[name] = _build_k1() if name == "k1" else _build_k2()
    return _progs[name]


def _run_k1(query, emb, trace=False):
    nc = _get("k1")
    q = np.ascontiguousarray(query, dtype=np.float32)
    in_maps = [
        {"q": q, "e": np.ascontiguousarray(emb[c * CSH:(c + 1) * CSH])}
        for c in range(NCORES)
    ]
    return run_bass_kernel_spmd(
        nc, in_maps, core_ids=list(range(NCORES)), trace=trace
    )


def _run_k2(in_maps, trace=False):
    nc = _get("k2")
    return run_bass_kernel_spmd(
        nc, in_maps, core_ids=list(range(NCORES)), trace=trace
    )


def kernel(query, episode_embeddings, episodes, compressed_episodes,
           is_compressed, k, _trace=False, _results=None):
    assert int(k) == K
    r1 = _run_k1(query, episode_embeddings, trace=_trace)

    # host: merge the 8 per-shard candidate lists -> global top-8
    cand_s = np.concatenate(
        [r1.results[c]["scores"] for c in range(NCORES)], axis=1
    )  # [B, 64]
    cand_i = np.concatenate(
        [r1.results[c]["idx"].astype(np.int64) + c * CSH for c in range(NCORES)],
        axis=1,
    )
    order = np.argsort(-cand_s, axis=1, kind="stable")[:, :K]
    top_scores = np.take_along_axis(cand_s, order, axis=1)
    top_idx = np.take_along_axis(cand_i, order, axis=1)  # [B, K] global

    # host: per-core gather/scatter index tensors
    comp = np.asarray(is_compressed).astype(bool)
    flat_idx = top_idx.reshape(-1)              # [B*K] winner slot w -> episode
    flat_comp = comp[flat_idx]
    owner = flat_idx // CSH
    ep_half = np.asarray(episodes, dtype=np.float32).reshape(C, 2, CROW)
    cp_flat = np.asarray(compressed_episodes, dtype=np.float32).reshape(C, CROW)

    in2 = []
    owned = []
    for c in range(NCORES):
        w = np.nonzero(owner == c)[0]           # winner slots owned by core c
        n = len(w)
        assert n <= CAP, f"core {c} owns {n} winners > capacity {CAP}"
        li = (flat_idx[w] - c * CSH).astype(np.int64)   # local table rows
        fc = flat_comp[w]
        # spread ranks across batches: j -> rank (j % NB)*128 + j//NB
        j = np.arange(n)
        rank = (j % NB) * 128 + j // NB
        owned.append((w, rank))
        OOBT = np.int32(3 * CSH)
        OOBS = np.int32(2 * CAP)
        g0v = np.full(CAP, OOBT, np.int32)
        g1v = np.full(CAP, OOBT, np.int32)
        s0v = np.full(CAP, OOBS, np.int32)
        s1v = np.full(CAP, OOBS, np.int32)
        g0v[rank] = np.where(fc, 2 * CSH + li, 2 * li)
        g1v[rank] = np.where(fc, OOBT, 2 * li + 1)
        s0v[rank] = 2 * rank
        s1v[rank] = np.where(fc, OOBS, 2 * rank + 1)
        tbl = np.concatenate(
            [ep_half[c * CSH:(c + 1) * CSH].reshape(2 * CSH, CROW),
             cp_flat[c * CSH:(c + 1) * CSH]], axis=0
        )
        # device tensors are [128, NB]: rank = bt*128 + p -> [p, bt]
        in2.append({
            "tbl": tbl,
            "g0": np.ascontiguousarray(g0v.reshape(NB, 128).T),
            "g1": np.ascontiguousarray(g1v.reshape(NB, 128).T),
            "slot0": np.ascontiguousarray(s0v.reshape(NB, 128).T),
            "slot1": np.ascontiguousarray(s1v.reshape(NB, 128).T),
        })

    r2 = _run_k2(in2, trace=_trace)

    retrieved = np.empty((B * K, S, H), dtype=np.float32)
    for c in range(NCORES):
        w, rank = owned[c]
        retrieved[w] = r2.results[c]["out"][rank].reshape(-1, S, H)
    retrieved = retrieved.reshape(B, K, S, H)

    if _results is not None:
        _results["r1"] = r1
        _results["r2"] = r2
    return retrieved, top_scores.astype(np.float32)


# revision 6
# speedup vs baseline: 1.3593x; 1.3593x over previous
"""Episodic-memory retrieval kernel for 8 Trainium2 NeuronCores.

Pipeline (classic sharded ANN retrieval, capacity-axis sharding):
  K1: per-core cosine sim (queries x embedding-shard) + local top-8
  host: merge candidates -> global top-8, build gather/scatter indices
  K2: per-core indirect-DMA gather of owned winners from episode shards,
      scatter into a compacted per-core output; host reassembles.
"""

import numpy as np

import concourse.bass as bass
import concourse.bacc as bacc
import concourse.mybir as mybir
from concourse.bass import IndirectOffsetOnAxis
from concourse.bass_utils import run_bass_kernel_spmd
from concourse.masks import make_identity
from concourse.tile import TileContext

F32 = mybir.dt.float32
I32 = mybir.dt.int32
U32 = mybir.dt.uint32

B, H, C, S, CS, K = 256, 256, 16384, 64, 32, 8
NCORES = 8
CSH = C // NCORES        # 2048 episodes per core shard
EPS = 1e-8
NB = 4                   # K2 batches of 128 winner-slots -> capacity 512/core
CAP = NB * 128
ROW = S * H              # 16384 f32 per full episode
CROW = CS * H            # 8192 f32 per compressed episode

_progs = {}


def _build_k1():
    nc = bacc.Bacc(None, target_bir_lowering=False)
    q = nc.dram_tensor("q", [B, H], F32, kind="ExternalInput")
    e = nc.dram_tensor("e", [CSH, H], F32, kind="ExternalInput")
    scores = nc.dram_tensor("scores", [B, K], F32, kind="ExternalOutput")
    idx = nc.dram_tensor("idx", [B, K], U32, kind="ExternalOutput")

    with TileContext(nc) as tc:
        with (
            tc.tile_pool(name="work", bufs=3) as wp,
            tc.tile_pool(name="psum", bufs=4, space="PSUM") as pp,
            tc.tile_pool(name="persist", bufs=1) as keep,
        ):
            ident = keep.tile([128, 128], F32, tag="ident")
            make_identity(nc, ident[:])

            def normalize(t):
                # t: [128, H] rows; t <- t / max(||t||, EPS) per row
                sq = wp.tile([128, H], F32, tag="sq")
                ssq = wp.tile([128, 1], F32, tag="ssq")
                nc.scalar.activation(
                    out=sq[:], in_=t[:],
                    func=mybir.ActivationFunctionType.Square,
                    accum_out=ssq[:],
                )
                nrm = wp.tile([128, 1], F32, tag="nrm")
                nc.scalar.activation(
                    out=nrm[:], in_=ssq[:],
                    func=mybir.ActivationFunctionType.Sqrt,
                )
                nc.vector.tensor_scalar_max(nrm[:], nrm[:], EPS)
                rn = wp.tile([128, 1], F32, tag="rn")
                nc.vector.reciprocal(rn[:], nrm[:])
                nc.vector.tensor_scalar_mul(t[:], t[:], rn[:])

            # qT[kt] : [128 (h), B] with h-slice kt
            qT = [keep.tile([128, B], F32, tag=f"qT{i}", name=f"qT{i}") for i in range(H // 128)]
            for mt in range(B // 128):
                qt = wp.tile([128, H], F32, tag="qt")
                nc.sync.dma_start(out=qt[:], in_=q[mt * 128:(mt + 1) * 128, :])
                normalize(qt)
                for kt in range(H // 128):
                    pt = pp.tile([128, 128], F32, space="PSUM", tag="pt")
                    nc.tensor.transpose(
                        out=pt[:], in_=qt[:, kt * 128:(kt + 1) * 128],
                        identity=ident[:],
                    )
                    nc.vector.tensor_copy(
                        out=qT[kt][:, mt * 128:(mt + 1) * 128], in_=pt[:]
                    )

            # eT[kt] : [128 (h), CSH]
            eT = [keep.tile([128, CSH], F32, tag=f"eT{i}", name=f"eT{i}") for i in range(H // 128)]
            for ct in range(CSH // 128):
                et = wp.tile([128, H], F32, tag="et")
                nc.sync.dma_start(out=et[:], in_=e[ct * 128:(ct + 1) * 128, :])
                normalize(et)
                for kt in range(H // 128):
                    pt = pp.tile([128, 128], F32, space="PSUM", tag="pt")
                    nc.tensor.transpose(
                        out=pt[:], in_=et[:, kt * 128:(kt + 1) * 128],
                        identity=ident[:],
                    )
                    nc.vector.tensor_copy(
                        out=eT[kt][:, ct * 128:(ct + 1) * 128], in_=pt[:]
                    )

            # sim[mt] : [128 (query), CSH], then top-8 per row
            for mt in range(B // 128):
                simt = keep.tile([128, CSH], F32, tag=f"sim{mt}", name=f"sim{mt}")
                for nt in range(CSH // 512):
                    ps = pp.tile([128, 512], F32, space="PSUM", tag="ps")
                    for kt in range(H // 128):
                        nc.tensor.matmul(
                            out=ps[:],
                            lhsT=qT[kt][:, mt * 128:(mt + 1) * 128],
                            rhs=eT[kt][:, nt * 512:(nt + 1) * 512],
                            start=(kt == 0),
                            stop=(kt == H // 128 - 1),
                        )
                    nc.vector.tensor_copy(
                        out=simt[:, nt * 512:(nt + 1) * 512], in_=ps[:]
                    )
                s8 = wp.tile([128, 8], F32, tag="s8")
                nc.vector.max(out=s8[:], in_=simt[:])
                i8 = wp.tile([128, 8], U32, tag="i8")
                nc.vector.max_index(out=i8[:], in_max=s8[:], in_values=simt[:])
                nc.sync.dma_start(
                    out=scores[mt * 128:(mt + 1) * 128, :], in_=s8[:]
                )
                nc.sync.dma_start(out=idx[mt * 128:(mt + 1) * 128, :], in_=i8[:])

    nc.compile()
    return nc


def _build_k2(scatter_queue=1):
    # tbl rows: [0, 2*CSH) = episode half-rows (2i, 2i+1), [2*CSH, 3*CSH) =
    # compressed rows. One gather per winner half, two independent
    # gather->scatter chains per batch. Scatters go on SWDGE queue 1 so the
    # SDMA engines round-robin between gather and scatter rings.
    nc = bacc.Bacc(None, target_bir_lowering=False,
                   num_swdge_queues=2 if scatter_queue else 1)
    tbl = nc.dram_tensor("tbl", [3 * CSH, CROW], F32, kind="ExternalInput")
    # idxs[:, t, b]: t = 0:g0 1:g1 2:slot0 3:slot1
    idxs = nc.dram_tensor("idxs", [128, 4, NB], I32, kind="ExternalInput")
    out = nc.dram_tensor("out", [CAP, ROW], F32, kind="ExternalOutput")
    out_half = out[:].rearrange("w (h x) -> (w h) x", h=2)  # [2*CAP, CROW]

    with TileContext(nc) as tc:
        with (
            tc.tile_pool(name="d0", bufs=3) as p0,
            tc.tile_pool(name="d1", bufs=3) as p1,
            tc.tile_pool(name="idxp", bufs=1) as ip,
        ):
            ix = ip.tile([128, 4, NB], I32, tag="ix")
            nc.sync.dma_start(out=ix[:], in_=idxs[:])

            for bt in range(NB):
                t0 = p0.tile([128, CROW], F32, tag="t0")
                t1 = p1.tile([128, CROW], F32, tag="t1")
                # half0 of every owned winner (episode half or compressed row)
                nc.gpsimd.indirect_dma_start(
                    out=t0[:],
                    out_offset=None,
                    in_=tbl[:],
                    in_offset=IndirectOffsetOnAxis(ap=ix[:, 0, bt:bt + 1], axis=0),
                    bounds_check=3 * CSH - 1,
                    oob_is_err=False,
                )
                # half1 (episode half-row 2i+1) for non-compressed winners
                nc.gpsimd.indirect_dma_start(
                    out=t1[:],
                    out_offset=None,
                    in_=tbl[:],
                    in_offset=IndirectOffsetOnAxis(ap=ix[:, 1, bt:bt + 1], axis=0),
                    bounds_check=3 * CSH - 1,
                    oob_is_err=False,
                )
                sc0 = nc.gpsimd.indirect_dma_start(
                    out=out_half,
                    out_offset=IndirectOffsetOnAxis(ap=ix[:, 2, bt:bt + 1], axis=0),
                    in_=t0[:],
                    in_offset=None,
                    bounds_check=2 * CAP - 1,
                    oob_is_err=False,
                )
                # compressed winners' half1 stays zero (pre-zeroed output)
                sc1 = nc.gpsimd.indirect_dma_start(
                    out=out_half,
                    out_offset=IndirectOffsetOnAxis(ap=ix[:, 3, bt:bt + 1], axis=0),
                    in_=t1[:],
                    in_offset=None,
                    bounds_check=2 * CAP - 1,
                    oob_is_err=False,
                )
                if scatter_queue:
                    sc0.ins.queue = f"qPoolDynamic{scatter_queue}"
                    sc1.ins.queue = f"qPoolDynamic{scatter_queue}"

    nc.compile()
    return nc


def _get(name):
    if name not in _progs:
        _progs[name] = _build_k1() if name == "k1" else _build_k2()
    return _progs[name]


def _run_k1(query, emb, trace=False):
    nc = _get("k1")
    q = np.ascontiguousarray(query, dtype=np.float32)
    in_maps = [
        {"q": q, "e": np.ascontiguousarray(emb[c * CSH:(c + 1) * CSH])}
        for c in range(NCORES)
    ]
    return run_bass_kernel_spmd(
        nc, in_maps, core_ids=list(range(NCORES)), trace=trace
    )


def _run_k2(in_maps, trace=False):
    nc = _get("k2")
    return run_bass_kernel_spmd(
        nc, in_maps, core_ids=list(range(NCORES)), trace=trace
    )


def kernel(query, episode_embeddings, episodes, compressed_episodes,
           is_compressed, k, _trace=False, _results=None):
    assert int(k) == K
    r1 = _run_k1(query, episode_embeddings, trace=_trace)

    # host: merge the 8 per-shard candidate lists -> global top-8
    cand_s = np.concatenate(
        [r1.results[c]["scores"] for c in range(NCORES)], axis=1
    )  # [B, 64]
    cand_i = np.concatenate(
        [r1.results[c]["idx"].astype(np.int64) + c * CSH for c in range(NCORES)],
        axis=1,
    )
    order = np.argsort(-cand_s, axis=1, kind="stable")[:, :K]
    top_scores = np.take_along_axis(cand_s, order, axis=1)
    top_idx = np.take_along_axis(cand_i, order, axis=1)  # [B, K] global

    # host: per-core gather/scatter index tensors
    comp = np.asarray(is_compressed).astype(bool)
    flat_idx = top_idx.reshape(-1)              # [B*K] winner slot w -> episode
    flat_comp = comp[flat_idx]
    owner = flat_idx // CSH
    ep_half = np.asarray(episodes, dtype=np.float32).reshape(C, 2, CROW)
    cp_flat = np.asarray(compressed_episodes, dtype=np.float32).reshape(C, CROW)

    in2 = []
    owned = []
    for c in range(NCORES):
        w = np.nonzero(owner == c)[0]           # winner slots owned by core c
        n = len(w)
        assert n <= CAP, f"core {c} owns {n} winners > capacity {CAP}"
        li = (flat_idx[w] - c * CSH).astype(np.int64)   # local table rows
        fc = flat_comp[w]
        # spread winners across batches, and within each batch interleave
        # the active rows evenly over all 128 partitions (SDMA engine balance)
        j = np.arange(n)
        b = j % NB
        i = j // NB
        mb = np.array([(n - bb + NB - 1) // NB for bb in range(NB)])
        mb = np.maximum(mb, 1)
        p = (i * 128) // mb[b]
        rank = b * 128 + p
        owned.append((w, rank))
        OOBT = np.int32(3 * CSH)
        OOBS = np.int32(2 * CAP)
        g0v = np.full(CAP, OOBT, np.int32)
        g1v = np.full(CAP, OOBT, np.int32)
        s0v = np.full(CAP, OOBS, np.int32)
        s1v = np.full(CAP, OOBS, np.int32)
        g0v[rank] = np.where(fc, 2 * CSH + li, 2 * li)
        g1v[rank] = np.where(fc, OOBT, 2 * li + 1)
        s0v[rank] = 2 * rank
        s1v[rank] = np.where(fc, OOBS, 2 * rank + 1)
        tbl = np.concatenate(
            [ep_half[c * CSH:(c + 1) * CSH].reshape(2 * CSH, CROW),
             cp_flat[c * CSH:(c + 1) * CSH]], axis=0
        )
        # device tensor idxs[p, t, bt]; rank = bt*128 + p
        iv = np.stack([g0v, g1v, s0v, s1v], axis=1).reshape(NB, 128, 4)
        in2.append({
            "tbl": tbl,
            "idxs": np.ascontiguousarray(iv.transpose(1, 2, 0)),
        })

    r2 = _run_k2(in2, trace=_trace)

    retrieved = np.empty((B * K, S, H), dtype=np.float32)
    for c in range(NCORES):
        w, rank = owned[c]
        retrieved[w] = r2.results[c]["out"][rank].reshape(-1, S, H)
    retrieved = retrieved.reshape(B, K, S, H)

    if _results is not None:
        _results["r1"] = r1
        _results["r2"] = r2
    return retrieved, top_scores.astype(np.float32)
